# revision 19
# baseline (speedup 1.0000x reference)
"""Trainium2 Bass kernel for AttentionFact:
    scores = einsum('bsh,ch->bcs', hidden, querys)
    factor = softmax(scores, axis=2)
    out    = einsum('bcs,bsh->bch', factor, hidden).reshape(B, C*H)

Shapes: B=16, S=4096, H=1024, C=64, fp32.

Strategy: data-parallel over batch. Each of the 8 NeuronCores handles 2
batches; querys (small) is replicated, pre-transposed on host into
[128, 8, 64] h-chunk layout. No collectives: host concatenates the
per-core [2, C*H] outputs.

v2 — PE column-tiling + fused softmax plumbing:
  - hidden streamed once per batch in 8 s-tiles of 512 rows with
    f32->fp16 cast during the DMA (SWDGE); fp16 natural-layout tiles
    stay resident in SBUF for the second einsum (nat bufs=16: the full
    16 MB working set is resident, the load queue never stalls)
  - hT blocks produced by PE transpose, evacuated PSUM->SBUF on DVE
    (3/4) + ScalarE (1/4)
  - mm1 is column-tiled over the two 256-col s-halves of each s-tile:
    both strips share the stationary qT[j] and stream their own hT
    columns concurrently in col-strips (0,0)/(0,64); M=64 would
    otherwise leave half the PE array idle
  - evacuation of each strip is one DVE tensor_tensor_reduce (in1 is an
    SBUF zeros tile: the verifier allows only one PSUM input): writes
    -(scores) into a [128, 2048] tile (c on partitions 0:64 for s <
    2048, 64:128 above) with a fused running min = -(running max)
  - softmax per s-half: ScalarE exp(scale=-1, bias=-max) with fused
    row-sum accumulation, writing fp16 factors
  - factor blocks PE-transposed to factorT [s, c]; mm2 is column-tiled
    over the two h-halves (strip0 -> h 0:512 into bank0 rows 0:64,
    strip1 -> h 512:1024 into bank1 rows 64:128), so the final
    normalize-evac reads a single PSUM region per op
  - row-sum normalization: micro PE transposes fold the per-half sums,
    DVE reciprocal, scale on evac, DMA out
"""

import numpy as np

import concourse.bass as bass
import concourse.mybir as mybir
import concourse.tile as tile
from concourse import bacc
from concourse.bass_utils import run_bass_kernel_spmd

B, S, H, C = 16, 4096, 1024, 64
NCORES = 8
BPC = B // NCORES          # batches per core
ST = 8                     # s-tiles per batch (512 rows each)
SQ = 4                     # 128-row subtiles per s-tile
HJ = H // 128              # h-chunks (8)

F32 = mybir.dt.float32
F16 = mybir.dt.float16
ADD = mybir.AluOpType.add
MIN = mybir.AluOpType.min
MAX = mybir.AluOpType.max
AXX = mybir.AxisListType.X
EXP = mybir.ActivationFunctionType.Exp
CPY = mybir.ActivationFunctionType.Copy

POS_BIG = 3.0e38


def build_nc():
    nc = bacc.Bacc("TRN2", target_bir_lowering=False, debug=False)
    hidden = nc.declare_dram_parameter("hidden", [BPC, S, H], F32, isOutput=False)
    qT = nc.declare_dram_parameter("qT", [128, HJ, C], F16, isOutput=False)
    ident = nc.declare_dram_parameter("ident", [128, 128], F16, isOutput=False)
    out = nc.declare_dram_parameter("out", [BPC, C, H], F32, isOutput=True)

    with tile.TileContext(nc) as tc:
        with (
            tc.tile_pool(name="const", bufs=1) as const_pool,
            tc.tile_pool(name="nat", bufs=16) as nat_pool,
            tc.tile_pool(name="hT", bufs=8) as hT_pool,
            tc.tile_pool(name="scores", bufs=2) as scores_pool,
            tc.tile_pool(name="expf", bufs=2) as exp_pool,
            tc.tile_pool(name="fT", bufs=2) as fT_pool,
            tc.tile_pool(name="stats", bufs=2) as stats_pool,
            tc.tile_pool(name="outp", bufs=2) as out_pool,
            tc.tile_pool(name="psT", bufs=3, space="PSUM") as psT_pool,
            tc.tile_pool(name="psS", bufs=1, space="PSUM") as psS_pool,
            tc.tile_pool(name="psF", bufs=2, space="PSUM") as psF_pool,
            tc.tile_pool(name="psR", bufs=2, space="PSUM") as psR_pool,
        ):
            ident_sb = const_pool.tile([128, 128], F16, tag="ident")
            nc.sync.dma_start(out=ident_sb[:], in_=ident[:])
            qT_sb = const_pool.tile([128, HJ, C], F16, tag="qT")
            nc.sync.dma_start(out=qT_sb[:], in_=qT[:])

            nat_tiles = {}
            scores_tiles = {}
            exp_tiles = {}
            rs_tiles = {}
            rm_state = {}      # (b, half) -> running -(max) tile (chain head)
            rm_final = {}      # (b, half) -> final -(max) tile
            psR_tiles = {}

            hT_tiles = {}

            def p1(b, st):
                """Load s-tile (2 half-DMAs), transpose to hT; on odd
                s-tiles run the col-tiled mm1 pair (strip0 = even tile,
                strip1 = odd tile, both N=512) + evac + running max."""
                nat_t = nat_pool.tile([128, SQ, H], F16, tag="nat")
                nat_tiles[(b, st)] = nat_t
                for hq in range(2):
                    src = hidden[b, st * 512 + hq * 256:
                                 st * 512 + (hq + 1) * 256, :].rearrange(
                        "(q p) h -> p q h", p=128
                    )
                    nc.gpsimd.dma_start(
                        out=nat_t[:, 2 * hq:2 * hq + 2, :], in_=src
                    )

                if st == 0:
                    scores_tiles[b] = scores_pool.tile(
                        [64, S], F32, tag="scores", name="scores"
                    )
                scores_sb = scores_tiles[b]

                for jp in range(4):
                    ps_t = psT_pool.tile([128, 1024], F16, tag="psT")
                    hT = hT_pool.tile([128, 1024], F16, tag="hT", bufs=10)
                    hT_tiles[(b, st, jp)] = hT
                    for ji in range(2):
                        j = jp * 2 + ji
                        for q in range(SQ):
                            nc.tensor.transpose(
                                ps_t[:, ji * 512 + q * 128:
                                     ji * 512 + (q + 1) * 128],
                                nat_t[:, q, j * 128:(j + 1) * 128],
                                ident_sb[:],
                            )
                    if jp % 2 == 1:
                        nc.scalar.copy(hT[:], ps_t[:])
                    else:
                        nc.vector.tensor_copy(hT[:], ps_t[:])

                if st % 2 == 0:
                    return

                # mm1 for the (st-1, st) pair: strip sh streams tile
                # st-1+sh's hT columns; both strips share stationary qT[j]
                ps_sc = psS_pool.tile([128, 512], F32, tag="psS")
                for jp in range(4):
                    for ji in range(2):
                        j = jp * 2 + ji
                        for sh in range(2):
                            nc.tensor.matmul(
                                ps_sc[sh * 64:(sh + 1) * 64, :],
                                qT_sb[:, j, :],
                                hT_tiles[(b, st - 1 + sh, jp)][
                                    :, ji * 512:(ji + 1) * 512],
                                start=(j == 0),
                                stop=(j == 7),
                                tile_position=(0, sh * 64),
                                skip_group_check=True,
                            )

                # evac strips into scores: DVE for strip 0 (tile st-1),
                # ScalarE for strip 1 (tile st); then fold into running max
                nc.vector.tensor_copy(
                    scores_sb[:, (st - 1) * 512:st * 512], ps_sc[0:64, :]
                )
                nc.scalar.copy(
                    scores_sb[:, st * 512:(st + 1) * 512],
                    ps_sc[64:128, :],
                )
                pm = stats_pool.tile([C, 1], F32, tag="pm", bufs=4)
                nc.vector.reduce_max(
                    pm[:], scores_sb[:, (st - 1) * 512:(st + 1) * 512],
                    axis=AXX
                )
                if st == 1:
                    rm_state[b] = pm
                else:
                    rmn = stats_pool.tile([C, 1], F32, tag="rm", bufs=8)
                    nc.vector.scalar_tensor_tensor(
                        out=rmn[:], in0=pm[:], scalar=0.0,
                        in1=rm_state[b][:], op0=ADD, op1=MAX,
                    )
                    rm_state[b] = rmn
                if st == ST - 1:
                    negmax = stats_pool.tile([C, 1], F32, tag="negmax")
                    nc.vector.tensor_scalar_mul(
                        negmax[:], rm_state[b][:], -1.0
                    )
                    rm_final[b] = negmax
                    exp_tiles[b] = exp_pool.tile([64, S], F16,
                                                 tag="expf", name="expf")
                    rs_tiles[b] = stats_pool.tile([C, ST], F32,
                                                  tag="rs", name="rs")
                    psR_tiles[b] = (
                        psR_pool.tile([128, 512], F32, tag="psR0",
                                      name="psR0", bufs=1),
                        psR_pool.tile([128, 512], F32, tag="psR1",
                                      name="psR1", bufs=1),
                    )

            def sm_chunk(b, ci):
                """exp chunk ci (512 cols) with fused row-sum accum."""
                nc.scalar.activation(
                    exp_tiles[b][:, ci * 512:(ci + 1) * 512],
                    scores_tiles[b][:, ci * 512:(ci + 1) * 512],
                    EXP,
                    bias=rm_final[b][:],
                    scale=1.0,
                    accum_out=rs_tiles[b][:, ci:ci + 1],
                )

            def p3_chunk(b, ci):
                """factorT transposes + col-tiled mm2 for chunk ci."""
                exp_sb = exp_tiles[b]
                ph = psR_tiles[b]
                idn = ident_sb[0:64, 0:64]
                ps_f = psF_pool.tile([128, 4 * C], F16, tag="psF")
                fTt = fT_pool.tile([128, 4 * C], F16, tag="fT")
                for ki in range(4):
                    nc.tensor.transpose(
                        ps_f[:, ki * C:(ki + 1) * C],
                        exp_sb[:, ci * 512 + ki * 128:
                               ci * 512 + (ki + 1) * 128],
                        idn,
                    )
                nc.vector.tensor_copy(fTt[:], ps_f[:])
                # col-tiled pairs: h-half 0 -> strip (0,0) bank0 rows 0:64,
                # h-half 1 -> strip (0,64) bank1 rows 64:128; both strips
                # share stationary fT[k]
                for ki in range(4):
                    k = ci * 4 + ki
                    natk = nat_tiles[(b, ci)]
                    fk = fTt[:, ki * C:(ki + 1) * C]
                    nc.tensor.matmul(
                        ph[0][0:64, :],
                        fk,
                        natk[:, ki, 0:512],
                        start=(k == 0),
                        stop=(k == 31),
                        tile_position=(0, 0),
                        skip_group_check=True,
                    )
                    nc.tensor.matmul(
                        ph[1][64:128, :],
                        fk,
                        natk[:, ki, 512:1024],
                        start=(k == 0),
                        stop=(k == 31),
                        tile_position=(0, 64),
                        skip_group_check=True,
                    )

            def finalize(b):
                """row-sum fold, reciprocal, scale on evac, DMA out."""
                rsum = stats_pool.tile([C, 1], F32, tag="rsum")
                nc.vector.reduce_sum(rsum[:], rs_tiles[b][:], axis=AXX)
                rinv = stats_pool.tile([C, 1], F32, tag="rinv")
                nc.vector.reciprocal(rinv[:], rsum[:])

                ph = psR_tiles[b]
                out_sb = out_pool.tile([C, H], F32, tag="out")
                nc.vector.tensor_scalar_mul(
                    out_sb[:, 0:512], ph[0][0:64, :], rinv[:]
                )
                nc.vector.tensor_scalar_mul(
                    out_sb[:, 512:1024], ph[1][64:128, :], rinv[:]
                )
                nc.sync.dma_start(out=out[b], in_=out_sb[:])

            # phase-interleaved schedule: batch 0 loads + mm1 first; its
            # softmax+mm2 chunks interleave with batch 1's s-tiles so the
            # PE never idles and the DMA queue stays saturated.  All of a
            # batch's exps are emitted before its p3 chunks so the Scalar
            # FIFO streams them back-to-back in the tail.
            for st in range(ST):
                p1(0, st)
            p1(1, 0)
            sm_chunk(0, 0); sm_chunk(0, 1)
            p3_chunk(0, 0); p3_chunk(0, 1)
            p1(1, 1)
            sm_chunk(0, 2); sm_chunk(0, 3)
            p3_chunk(0, 2); p3_chunk(0, 3)
            p1(1, 2)
            sm_chunk(0, 4); sm_chunk(0, 5)
            p3_chunk(0, 4); p3_chunk(0, 5)
            p1(1, 3)
            sm_chunk(0, 6); sm_chunk(0, 7)
            p3_chunk(0, 6); p3_chunk(0, 7)
            finalize(0)
            p1(1, 4); p1(1, 5); p1(1, 6); p1(1, 7)
            for ci in range(ST):
                sm_chunk(1, ci)
            for ci in range(ST):
                p3_chunk(1, ci)
            finalize(1)

    nc.compile()
    return nc


_NC_CACHE = None


def _get_nc():
    global _NC_CACHE
    if _NC_CACHE is None:
        _NC_CACHE = build_nc()
    return _NC_CACHE


def kernel(hidden, querys):
    hidden = np.ascontiguousarray(np.asarray(hidden), dtype=np.float32)
    querys = np.ascontiguousarray(np.asarray(querys), dtype=np.float32)
    assert hidden.shape == (B, S, H) and querys.shape == (C, H)

    # qT[k, j, c] = querys[c, j*128 + k]  (h-chunk-major transposed layout)
    qT = np.ascontiguousarray(
        querys.T.reshape(HJ, 128, C).transpose(1, 0, 2)
    ).astype(np.float16)
    ident = np.eye(128, dtype=np.float16)

    nc = _get_nc()
    in_maps = [
        {
            "hidden": np.ascontiguousarray(hidden[i * BPC:(i + 1) * BPC]),
            "qT": qT,
            "ident": ident,
        }
        for i in range(NCORES)
    ]
    res = run_bass_kernel_spmd(nc, in_maps, core_ids=list(range(NCORES)))
    global LAST_RESULTS
    LAST_RESULTS = res
    outs = [np.asarray(res.results[i]["out"]).reshape(BPC, C * H)
            for i in range(NCORES)]
    return np.concatenate(outs, axis=0)


LAST_RESULTS = None
